# revision 1
# baseline (speedup 1.0000x reference)
"""Trainium2 Bass kernel for nn_LocalDenseCrossReadout.

Strategy:
- Data-parallel over batch: 8 batches -> 8 NeuronCores, one batch per core.
- Host-side (numpy, ~0.1% of FLOPs): FiLM conditioning (ctx -> gamma/beta),
  folding of LayerNorm affine + FiLM + score scale into the projection
  weights, and band-slicing of the additive mask.
- Device kernel per core: LayerNorm stats+apply for q [1024,512] and
  source [4096,512], transposed projections (f32r matmuls), banded local
  attention (768-wide aligned kv window per 128-row q tile), low-rank gate
  bias, softmax, attn@V and output projection.
"""

import sys

sys.path.insert(0, "/opt/trn_rl_repo")

import numpy as np

import concourse.bass as bass
import concourse.tile as tile
from concourse import bacc
from concourse import mybir
from concourse.bass_utils import run_bass_kernel_spmd
from concourse.masks import make_identity

DIM, QS, QT, KS, KT, WIN, B, RANK = 512, 64, 16, 256, 16, 4, 8, 32
Q = QS * QT  # 1024
K = KS * KT  # 4096
WINW = 768  # aligned kv window per 128-row q tile
NQT = Q // 128  # 8 q tiles
F32 = mybir.dt.float32
F32R = mybir.dt.float32r
FT = mybir.ActivationFunctionType
ALU = mybir.AluOpType
AX = mybir.AxisListType

# kv window start (aligned to 128) per q tile; phase split of the kv axis
WSTARTS = [0, 384, 896, 1408, 1920, 2432, 2944, 3328]
PHASES = [  # (kv_start, n_kv_tiles, q_tiles)
    (0, 17, range(0, 4)),
    (1920, 17, range(4, 8)),
]
KVW = 17 * 128  # 2176 kv columns held on-chip per phase


def r32(ap):
    return ap.bitcast(F32R)


def build_bass(debug=False, stage=5):
    nc = bacc.Bacc("TRN2", target_bir_lowering=False)
    q = nc.dram_tensor("q", [Q, DIM], F32, kind="ExternalInput")
    s = nc.dram_tensor("s", [K, DIM], F32, kind="ExternalInput")
    wq = nc.dram_tensor("wq", [DIM, DIM], F32R, kind="ExternalInput")
    wk = nc.dram_tensor("wk", [DIM, DIM], F32R, kind="ExternalInput")
    wv = nc.dram_tensor("wv", [DIM, DIM], F32R, kind="ExternalInput")
    wo = nc.dram_tensor("wo", [DIM, DIM], F32R, kind="ExternalInput")
    wgq = nc.dram_tensor("wgq", [DIM, RANK], F32R, kind="ExternalInput")
    wgk = nc.dram_tensor("wgk", [DIM, RANK], F32R, kind="ExternalInput")
    rqt = nc.dram_tensor("rqt", [128, 4], F32, kind="ExternalInput")
    rkt = nc.dram_tensor("rkt", [128, 4], F32, kind="ExternalInput")
    rv = nc.dram_tensor("rv", [1, DIM], F32R, kind="ExternalInput")
    bo = nc.dram_tensor("bo", [1, DIM], F32R, kind="ExternalInput")
    bmask = nc.dram_tensor("bmask", [NQT, 128, WINW], F32, kind="ExternalInput")
    out = nc.dram_tensor("out", [Q, DIM], F32, kind="ExternalOutput")
    if debug:
        d_qpT = nc.dram_tensor("d_qpT", [128, 4, Q], F32, kind="ExternalOutput")
        d_gq = nc.dram_tensor("d_gq", [32, Q], F32, kind="ExternalOutput")
        d_kT = nc.dram_tensor("d_kT", [128, 4, KVW], F32, kind="ExternalOutput")
        d_vb = nc.dram_tensor("d_vb", [128, 17, DIM], F32, kind="ExternalOutput")
        d_gk = nc.dram_tensor("d_gk", [32, KVW], F32, kind="ExternalOutput")
        d_S = nc.dram_tensor("d_S", [128, WINW], F32, kind="ExternalOutput")
        d_P = nc.dram_tensor("d_P", [128, WINW], F32, kind="ExternalOutput")
        d_oa = nc.dram_tensor("d_oa", [128, DIM], F32, kind="ExternalOutput")

    with tile.TileContext(nc) as tc:
        with (
            tc.tile_pool(name="consts", bufs=1) as consts,
            tc.tile_pool(name="wts", bufs=1) as wts,
            tc.tile_pool(name="kv", bufs=1) as kvpool,
            tc.tile_pool(name="xin", bufs=3) as xin,
            tc.tile_pool(name="stats", bufs=4) as stats,
            tc.tile_pool(name="xt", bufs=2) as xtp,
            tc.tile_pool(name="attn", bufs=2) as attn,
            tc.tile_pool(name="outp", bufs=2) as outp,
            tc.tile_pool(name="ps_s", bufs=3, space="PSUM") as ps_s,
            tc.tile_pool(name="ps_b", bufs=2, space="PSUM") as ps_b,
        ):
            # ---------------- constants ----------------
            ident = consts.tile([128, 128], F32)
            make_identity(nc, ident)
            eps = consts.tile([128, 1], F32)
            nc.vector.memset(eps, 1e-5)
            ones1 = consts.tile([1, 128], F32R)
            nc.vector.memset(ones1.bitcast(F32), 1.0)
            zero_c = consts.tile([128, 1], F32)
            nc.vector.memset(zero_c, 0.0)
            eps6 = consts.tile([128, 1], F32)
            nc.vector.memset(eps6, 1e-6)
            rqt_sb = consts.tile([128, 4], F32)
            nc.sync.dma_start(out=rqt_sb, in_=rqt[:, :])
            rkt_sb = consts.tile([128, 4], F32)
            nc.sync.dma_start(out=rkt_sb, in_=rkt[:, :])
            rv_sb = consts.tile([1, DIM], F32R)
            nc.sync.dma_start(out=rv_sb, in_=rv[:, :])
            bo_sb = consts.tile([1, DIM], F32R)
            nc.sync.dma_start(out=bo_sb, in_=bo[:, :])

            # weights as lhsT chunks: [128 (d_in in chunk c), c, d_out]
            def load_w(name, dram, n_out):
                t = wts.tile([128, 4, n_out], F32R, tag=name)
                for c in range(4):
                    nc.sync.dma_start(out=t[:, c, :], in_=dram[c * 128:(c + 1) * 128, :])
                return t

            wq_sb = load_w("wq", wq, DIM)
            wk_sb = load_w("wk", wk, DIM)
            wv_sb = load_w("wv", wv, DIM)
            wgq_sb = load_w("wgq", wgq, RANK)
            wgk_sb = load_w("wgk", wgk, RANK)

            # persistent activations
            qpT = kvpool.tile([128, 4, Q], F32R, tag="qpT")     # q_p^T chunks
            gq_sb = kvpool.tile([32, Q], F32R, tag="gq")        # gate_q^T

            # ---- LN + transpose of one 128-row tile into xt_big[:, :, j*128:] ----
            def ln_transpose(src_dram, row0, nrows, xt_big, jcol):
                x = xin.tile([128, DIM], F32, tag="x")
                nc.sync.dma_start(out=x[:nrows, :], in_=src_dram[row0:row0 + nrows, :])
                st6 = stats.tile([128, 6], F32, tag="st6")
                nc.vector.bn_stats(out=st6[:nrows], in_=x[:nrows, :])
                mv = stats.tile([128, 2], F32, tag="mv")
                nc.vector.bn_aggr(out=mv[:nrows], in_=st6[:nrows])
                sd = stats.tile([128, 1], F32, tag="sd")
                nc.scalar.activation(out=sd[:nrows], in_=mv[:nrows, 1:2],
                                     func=FT.Sqrt, bias=eps[:nrows], scale=1.0)
                rstd = stats.tile([128, 1], F32, tag="rstd")
                nc.vector.reciprocal(out=rstd[:nrows], in_=sd[:nrows])
                nmr = stats.tile([128, 1], F32, tag="nmr")
                nc.vector.scalar_tensor_tensor(
                    out=nmr[:nrows], in0=mv[:nrows, 0:1], scalar=-1.0,
                    in1=rstd[:nrows], op0=ALU.mult, op1=ALU.mult)
                xn = xin.tile([128, DIM], F32, tag="xn")
                nc.vector.tensor_scalar_mul(xn[:nrows], x[:nrows, :], rstd[:nrows])
                nc.vector.tensor_scalar_add(xn[:nrows], xn[:nrows], nmr[:nrows])
                tp = ps_s.tile([128, 4, 128], F32, tag="ps")
                for c in range(4):
                    nc.tensor.transpose(tp[:, c, :nrows], xn[:nrows, c * 128:(c + 1) * 128], ident)
                nc.vector.tensor_copy(xt_big[:, :, jcol * 128:jcol * 128 + nrows], tp[:, :, :nrows])

            # ---------------- phase A: queries ----------------
            for sup in range(2):  # 512 q rows each
                qt_big = xtp.tile([128, 4, 512], F32R, tag="xt_big")
                for j in range(4):
                    ln_transpose(q, sup * 512 + j * 128, 128, qt_big, j)
                # q_p^T chunks for these 512 q columns
                for m in range(4):
                    pp = ps_s.tile([128, 512], F32, tag="ps")
                    for c in range(4):
                        nc.tensor.matmul(pp, r32(wq_sb[:, c, m * 128:(m + 1) * 128]),
                                         r32(qt_big[:, c, :]), start=(c == 0), stop=(c == 3))
                    nc.scalar.activation(out=qpT[:, m, sup * 512:(sup + 1) * 512], in_=pp,
                                         func=FT.Identity, bias=rqt_sb[:, m:m + 1], scale=1.0)
                # gate_q^T = WgqS^T @ q_p^T (contraction over q_p feature dim)
                gp = ps_s.tile([32, 512], F32, tag="ps_g", bufs=1)
                for c in range(4):
                    nc.tensor.matmul(gp, r32(wgq_sb[:, c, :]),
                                     qpT[:, c, sup * 512:(sup + 1) * 512],
                                     start=(c == 0), stop=(c == 3))
                nc.vector.tensor_copy(gq_sb[:, sup * 512:(sup + 1) * 512], gp)

            # wo shares wq's slot; loaded after last wq use (phase A done)
            wo_sb = load_w("wq", wo, DIM)

            # ---------------- kv phases ----------------
            for kv_start, n_kv, q_tiles in PHASES:
                kT = kvpool.tile([128, 4, KVW], F32R, tag="kT")
                vb = kvpool.tile([128, 17, DIM], F32R, tag="vb")
                gk_sb = kvpool.tile([32, KVW], F32R, tag="gk")

                for sup in range(5):  # supertiles of 4,4,4,4,1 kv tiles
                    j0 = sup * 4
                    nt = min(4, n_kv - j0)
                    ncols = nt * 128
                    st_big = xtp.tile([128, 4, 512], F32R, tag="xt_big")
                    for j in range(nt):
                        ln_transpose(s, kv_start + (j0 + j) * 128, 128, st_big, j)
                    # k_p^T chunks
                    for m in range(4):
                        pp = ps_s.tile([128, 512], F32, tag="ps")
                        for c in range(4):
                            nc.tensor.matmul(pp[:, :ncols], r32(wk_sb[:, c, m * 128:(m + 1) * 128]),
                                             r32(st_big[:, c, :ncols]), start=(c == 0), stop=(c == 3))
                        nc.scalar.activation(out=kT[:, m, j0 * 128:j0 * 128 + ncols], in_=pp[:, :ncols],
                                             func=FT.Identity, bias=rkt_sb[:, m:m + 1], scale=1.0)
                    # v_p natural rows
                    for j in range(nt):
                        pv = ps_s.tile([128, 512], F32, tag="ps")
                        for c in range(4):
                            nc.tensor.matmul(pv, r32(st_big[:, c, j * 128:(j + 1) * 128]),
                                             r32(wv_sb[:, c, :]), start=(c == 0), stop=False)
                        nc.tensor.matmul(pv, r32(ones1), r32(rv_sb), start=False, stop=True)
                        nc.scalar.copy(vb[:, j0 + j, :], pv)
                    # gate_k^T = Wgk^T @ k_p^T
                    gp = ps_s.tile([32, 512], F32, tag="ps_g", bufs=1)
                    for c in range(4):
                        nc.tensor.matmul(gp[:, :ncols], r32(wgk_sb[:, c, :]),
                                         kT[:, c, j0 * 128:j0 * 128 + ncols],
                                         start=(c == 0), stop=(c == 3))
                    nc.vector.tensor_copy(gk_sb[:, j0 * 128:j0 * 128 + ncols], gp[:, :ncols])

                if debug and kv_start == 0:
                    nc.sync.dma_start(out=d_kT[:, :, :], in_=kT[:, :, :].bitcast(F32))
                    nc.sync.dma_start(out=d_vb[:, :, :], in_=vb[:, :, :].bitcast(F32))
                    nc.sync.dma_start(out=d_gk[:, :], in_=gk_sb[:, :].bitcast(F32))
                    nc.sync.dma_start(out=d_qpT[:, :, :], in_=qpT[:, :, :].bitcast(F32))
                    nc.sync.dma_start(out=d_gq[:, :], in_=gq_sb[:, :].bitcast(F32))

                # ---------------- attention over this phase's q tiles ----------------
                for t in q_tiles:
                    if stage < 3:
                        ob0 = outp.tile([128, DIM], F32, tag="ob")
                        nc.vector.tensor_copy(ob0, vb[:, 0, :].bitcast(F32))
                        nc.sync.dma_start(out=out[t * 128:(t + 1) * 128, :], in_=ob0)
                        continue
                    w0 = WSTARTS[t]
                    rel = w0 - kv_start
                    qc = bass.ts(t, 128)
                    msk = attn.tile([128, WINW], F32, tag="msk")
                    nc.sync.dma_start(out=msk, in_=bmask[t, :, :])
                    # gate logits -> gate bias
                    gl = ps_b.tile([128, WINW], F32, tag="ps_big")
                    for n0 in (0, 512):
                        nn_ = min(512, WINW - n0)
                        nc.tensor.matmul(gl[:, n0:n0 + nn_], r32(gq_sb[:, qc]),
                                         r32(gk_sb[:, rel + n0:rel + n0 + nn_]),
                                         start=True, stop=True)
                    if stage == 30:
                        obx = outp.tile([128, DIM], F32, tag="ob")
                        nc.vector.tensor_copy(obx, gl[:, :DIM])
                        nc.sync.dma_start(out=out[t * 128:(t + 1) * 128, :], in_=obx)
                        continue
                    sig = attn.tile([128, WINW], F32, tag="sig")
                    nc.scalar.activation(out=sig, in_=gl, func=FT.Sigmoid, bias=zero_c)
                    gb = attn.tile([128, WINW], F32, tag="gb")
                    nc.scalar.activation(out=gb, in_=sig, func=FT.Ln, bias=eps6, scale=1.0)
                    if stage == 31:
                        obx = outp.tile([128, DIM], F32, tag="ob")
                        nc.vector.tensor_copy(obx, gb[:, :DIM])
                        nc.sync.dma_start(out=out[t * 128:(t + 1) * 128, :], in_=obx)
                        continue
                    # scores
                    sc = ps_b.tile([128, WINW], F32, tag="ps_big")
                    for n0 in (0, 512):
                        nn_ = min(512, WINW - n0)
                        for c in range(4):
                            nc.tensor.matmul(sc[:, n0:n0 + nn_], r32(qpT[:, c, qc]),
                                             r32(kT[:, c, rel + n0:rel + n0 + nn_]),
                                             start=(c == 0), stop=(c == 3))
                    if stage == 32:
                        obx = outp.tile([128, DIM], F32, tag="ob")
                        nc.vector.tensor_copy(obx, sc[:, :DIM])
                        nc.sync.dma_start(out=out[t * 128:(t + 1) * 128, :], in_=obx)
                        continue
                    S = attn.tile([128, WINW], F32, tag="S")
                    nc.vector.scalar_tensor_tensor(out=S, in0=sc, scalar=1.0, in1=msk,
                                                   op0=ALU.mult, op1=ALU.add)
                    SG = attn.tile([128, WINW], F32, tag="sig")
                    nc.vector.tensor_add(SG, S, gb)
                    mx = stats.tile([128, 1], F32, tag="mx")
                    nc.vector.tensor_reduce(out=mx, in_=SG, axis=AX.X, op=ALU.max)
                    nmx = stats.tile([128, 1], F32, tag="nmx")
                    nc.vector.tensor_scalar_mul(nmx, mx, -1.0)
                    if debug and t == 0:
                        nc.sync.dma_start(out=d_S[:, :], in_=SG)
                    P = attn.tile([128, WINW], F32, tag="P")
                    nc.scalar.activation(out=P, in_=SG, func=FT.Exp, bias=nmx, scale=1.0)
                    rsum = stats.tile([128, 1], F32, tag="rsum")
                    nc.vector.tensor_reduce(out=rsum, in_=P, axis=AX.X, op=ALU.add)
                    rinv = stats.tile([128, 1], F32, tag="rinv")
                    nc.vector.reciprocal(out=rinv, in_=rsum)
                    if stage < 4 or stage == 33:
                        ob1 = outp.tile([128, DIM], F32, tag="ob")
                        nc.vector.tensor_copy(ob1, P[:, :DIM])
                        nc.sync.dma_start(out=out[t * 128:(t + 1) * 128, :], in_=ob1)
                        continue
                    # attn^T (unnormalized)
                    pt = ps_b.tile([128, WINW], F32, tag="ps_big")
                    for cc in range(6):
                        nc.tensor.transpose(pt[:, cc * 128:(cc + 1) * 128],
                                            P[:, cc * 128:(cc + 1) * 128], ident)
                    aT = attn.tile([128, 6, 128], F32R, tag="aT")
                    nc.vector.tensor_copy(aT, pt.rearrange("p (a b) -> p a b", a=6))
                    # attn @ V
                    av = ps_s.tile([128, 512], F32, tag="ps")
                    for cc in range(6):
                        nc.tensor.matmul(av, r32(aT[:, cc, :]), r32(vb[:, rel // 128 + cc, :]),
                                         start=(cc == 0), stop=(cc == 5))
                    oa = outp.tile([128, DIM], F32, tag="oa")
                    nc.vector.tensor_scalar_mul(oa, av, rinv)  # normalize rows
                    if debug and t == 0:
                        nc.sync.dma_start(out=d_P[:, :], in_=P)
                        nc.sync.dma_start(out=d_oa[:, :], in_=oa)
                    if stage < 5:
                        nc.sync.dma_start(out=out[t * 128:(t + 1) * 128, :], in_=oa)
                        continue
                    # out = oa @ Wo + bo
                    ot = ps_s.tile([128, 4, 128], F32, tag="ps")
                    for c in range(4):
                        nc.tensor.transpose(ot[:, c, :], oa[:, c * 128:(c + 1) * 128], ident)
                    oaT = outp.tile([128, 4, 128], F32R, tag="oaT")
                    nc.vector.tensor_copy(oaT, ot)
                    fin = ps_s.tile([128, 512], F32, tag="ps")
                    for c in range(4):
                        nc.tensor.matmul(fin, r32(oaT[:, c, :]), r32(wo_sb[:, c, :]),
                                         start=(c == 0), stop=False)
                    nc.tensor.matmul(fin, r32(ones1), r32(bo_sb), start=False, stop=True)
                    ob = outp.tile([128, DIM], F32, tag="ob")
                    nc.vector.tensor_copy(ob, fin)
                    nc.sync.dma_start(out=out[t * 128:(t + 1) * 128, :], in_=ob)

    if not nc.is_finalized():
        nc.finalize()
    return nc


_NC_CACHE = None


def _get_nc():
    global _NC_CACHE
    if _NC_CACHE is None:
        _NC_CACHE = build_bass()
    return _NC_CACHE


def _host_fold(inputs):
    f32 = np.float32
    scale = f32(DIM ** -0.5)
    ctx0 = np.asarray(inputs["ctx0"], f32)
    ctx1 = np.asarray(inputs["ctx1"], f32)
    pre = ctx0 @ inputs["Wc0"] + inputs["bc0"] + ctx1 @ inputs["Wc1"] + inputs["bc1"]
    pre = np.asarray(pre, f32)
    h = pre / (1.0 + np.exp(-pre))
    gb = np.asarray(h @ inputs["Wf"] + inputs["bf"], f32)
    gamma, beta = gb[:, :DIM], gb[:, DIM:]

    qn_g = np.asarray(inputs["qn_g"], f32)
    qn_b = np.asarray(inputs["qn_b"], f32)
    kvn_g = np.asarray(inputs["kvn_g"], f32)
    kvn_b = np.asarray(inputs["kvn_b"], f32)
    Wq, bq = np.asarray(inputs["Wq"], f32), np.asarray(inputs["bq"], f32)
    Wk, bk = np.asarray(inputs["Wk"], f32), np.asarray(inputs["bk"], f32)
    Wv, bv = np.asarray(inputs["Wv"], f32), np.asarray(inputs["bv"], f32)
    mask = np.asarray(inputs["mask"], f32)

    WkS = np.ascontiguousarray((Wk * kvn_g[:, None]).astype(f32))
    r_k = (kvn_b @ Wk + bk).astype(f32)
    WvS = np.ascontiguousarray((Wv * kvn_g[:, None]).astype(f32))
    r_v = (kvn_b @ Wv + bv).astype(f32)
    WgqS = np.ascontiguousarray((inputs["Wgq"] / scale / np.sqrt(RANK)).astype(f32))
    Wgk = np.ascontiguousarray(np.asarray(inputs["Wgk"], f32))
    Wo = np.ascontiguousarray(np.asarray(inputs["Wo"], f32))
    bo = np.asarray(inputs["bo"], f32)

    bmask = np.stack([mask[t * 128:(t + 1) * 128, w:w + WINW]
                      for t, w in enumerate(WSTARTS)]).astype(f32)
    bmask = np.ascontiguousarray(np.maximum(bmask, -1e30))  # avoid -inf on device

    query = np.asarray(inputs["query"], f32).reshape(B, Q, DIM)
    source = np.asarray(inputs["source"], f32).reshape(B, K, DIM)

    in_maps = []
    for b in range(B):
        sg = (qn_g * (1.0 + gamma[b])).astype(f32)
        WqS = np.ascontiguousarray((Wq * sg[:, None] * scale).astype(f32))
        r_q = (((qn_b * (1.0 + gamma[b]) + beta[b]) @ Wq + bq) * scale).astype(f32)
        in_maps.append({
            "q": np.ascontiguousarray(query[b]),
            "s": np.ascontiguousarray(source[b]),
            "wq": WqS, "wk": WkS, "wv": WvS, "wo": Wo,
            "wgq": WgqS, "wgk": Wgk,
            "rqt": np.ascontiguousarray(r_q.reshape(4, 128).T),
            "rkt": np.ascontiguousarray(r_k.reshape(4, 128).T),
            "rv": r_v.reshape(1, DIM),
            "bo": bo.reshape(1, DIM),
            "bmask": bmask,
        })
    return in_maps


def kernel(**inputs):
    nc = _get_nc()
    in_maps = _host_fold(inputs)
    res = run_bass_kernel_spmd(nc, in_maps, core_ids=list(range(B)))
    out = np.stack([res.results[b]["out"] for b in range(B)])
    return out.reshape(B, QS, QT, DIM).astype(np.float32)


if __name__ == "__main__":
    build_bass()
    print("bass build OK")



# revision 13
# speedup vs baseline: 1.5115x; 1.5115x over previous
"""Trainium2 Bass kernel for nn_LocalDenseCrossReadout (v2).

Strategy:
- Data-parallel over batch: 8 batches -> 8 NeuronCores, one batch per core.
- Host-side (numpy, O(D^2)): FiLM conditioning, folding of LayerNorm affine +
  FiLM + score scale into projection weights, v-bias folded into output bias,
  multiplicative 0/1 mask blocks.
- Device, all bf16 matmul inputs (fp32 PSUM accumulate):
  * LayerNorm stats+apply on natural-layout tiles (DVE, fused tensor_scalar)
  * x^T via hardware DMA-transpose (no PE transposes at all)
  * projections q/k/v/gates with weights stationary (FWL), biases applied in
    the PSUM->SBUF activation copy (per-partition, free)
  * banded attention, 768-wide window per 128-row q tile:
    gate sigma via tanh (same act table as exp -> no table reloads),
    P = exp(S) * (sigma * mask01) with fused row-sum (tensor_tensor_reduce),
    no max-subtraction, P^T and oa^T via DMA-transpose,
    softmax normalization folded into the activation-copy scale,
    v-bias + out-bias as one rank-1 matmul.
"""

import sys

sys.path.insert(0, "/opt/trn_rl_repo")

import numpy as np
import ml_dtypes

import concourse.bass as bass
import concourse.tile as tile
from concourse import bacc
from concourse import mybir
from concourse.bass_utils import run_bass_kernel_spmd

DIM, QS, QT, KS, KT, WIN, B, RANK = 512, 64, 16, 256, 16, 4, 8, 32
Q = QS * QT  # 1024
K = KS * KT  # 4096
WINW = 768  # aligned kv window per 128-row q tile
NQT = Q // 128  # 8 q tiles
NKT = K // 128  # 32 kv tiles
F32 = mybir.dt.float32
BF16 = mybir.dt.bfloat16
FT = mybir.ActivationFunctionType
ALU = mybir.AluOpType

BFNP = ml_dtypes.bfloat16

# kv window start (aligned to 128) per q tile
WSTARTS = [0, 384, 896, 1408, 1920, 2432, 2944, 3328]


def build_bass(debug=False, stage=4):
    nc = bacc.Bacc("TRN2", target_bir_lowering=False)
    q = nc.dram_tensor("q", [128, NQT, DIM], BF16, kind="ExternalInput")
    s = nc.dram_tensor("s", [128, NKT, DIM], BF16, kind="ExternalInput")
    wq = nc.dram_tensor("wq", [DIM, DIM], BF16, kind="ExternalInput")
    wk = nc.dram_tensor("wk", [DIM, DIM], BF16, kind="ExternalInput")
    wv = nc.dram_tensor("wv", [DIM, DIM], BF16, kind="ExternalInput")
    wo = nc.dram_tensor("wo", [DIM, DIM], BF16, kind="ExternalInput")
    wgq = nc.dram_tensor("wgq", [DIM, RANK], BF16, kind="ExternalInput")
    wgk = nc.dram_tensor("wgk", [DIM, RANK], BF16, kind="ExternalInput")
    rqt = nc.dram_tensor("rqt", [128, 4], F32, kind="ExternalInput")
    rkt = nc.dram_tensor("rkt", [128, 4], F32, kind="ExternalInput")
    bos = nc.dram_tensor("bos", [1, DIM], BF16, kind="ExternalInput")
    m01 = nc.dram_tensor("m01", [NQT, 128, WINW], BF16, kind="ExternalInput")
    out = nc.dram_tensor("out", [Q, DIM], F32, kind="ExternalOutput")
    if debug:
        d_qpT = nc.dram_tensor("d_qpT", [128, 4, Q], BF16, kind="ExternalOutput")
        d_kT = nc.dram_tensor("d_kT", [128, 4, K], BF16, kind="ExternalOutput")
        d_vb = nc.dram_tensor("d_vb", [128, NKT, DIM], BF16, kind="ExternalOutput")
        d_gq = nc.dram_tensor("d_gq", [32, Q], BF16, kind="ExternalOutput")
        d_gk = nc.dram_tensor("d_gk", [32, K], BF16, kind="ExternalOutput")
        d_P = nc.dram_tensor("d_P", [128, WINW], BF16, kind="ExternalOutput")
        d_oa = nc.dram_tensor("d_oa", [128, DIM], BF16, kind="ExternalOutput")

    with tile.TileContext(nc) as tc:
        with (
            tc.tile_pool(name="consts", bufs=1) as consts,
            tc.tile_pool(name="wts", bufs=1) as wts,
            tc.tile_pool(name="big", bufs=1) as bigp,
            tc.tile_pool(name="xin", bufs=2) as xin,
            tc.tile_pool(name="xt", bufs=2) as xtp,
            tc.tile_pool(name="stats", bufs=2) as stats,
            tc.tile_pool(name="attn", bufs=2) as attn,
            tc.tile_pool(name="outp", bufs=2) as outp,
            tc.tile_pool(name="ps_p", bufs=2, space="PSUM") as ps_p,
            tc.tile_pool(name="ps_b", bufs=2, space="PSUM") as ps_b,
            tc.tile_pool(name="ps_g", bufs=1, space="PSUM") as ps_g,
        ):
            # ---------------- constants ----------------
            eps = consts.tile([128, 1], F32)
            nc.vector.memset(eps, 1e-5)
            ones1 = consts.tile([1, 128], BF16)
            nc.vector.memset(ones1, 1.0)
            rqt_sb = consts.tile([128, 4], F32)
            nc.gpsimd.dma_start(out=rqt_sb, in_=rqt[:, :])
            rkt_sb = consts.tile([128, 4], F32)
            nc.gpsimd.dma_start(out=rkt_sb, in_=rkt[:, :])
            bos_sb = consts.tile([1, DIM], BF16)
            nc.gpsimd.dma_start(out=bos_sb, in_=bos[:, :])

            # weights as lhsT chunks: [128 (d_in within chunk c), c, d_out]
            def load_w(name, dram, n_out):
                t = wts.tile([128, 4, n_out], BF16, tag=name)
                for c in range(4):
                    nc.gpsimd.dma_start(out=t[:, c, :], in_=dram[c * 128:(c + 1) * 128, :])
                return t

            wq_sb = load_w("wq", wq, DIM)
            wk_sb = load_w("wk", wk, DIM)
            wv_sb = load_w("wv", wv, DIM)
            wo_sb = load_w("wo", wo, DIM)
            wgq_sb = load_w("wgq", wgq, RANK)
            wgk_sb = load_w("wgk", wgk, RANK)

            # persistent activation tensors
            qpT = bigp.tile([128, 4, Q], BF16, tag="qpT")     # q_p^T d-chunks
            kTp = bigp.tile([128, 4, K], BF16, tag="kTp")     # k_p^T d-chunks
            vb = bigp.tile([128, NKT, DIM], BF16, tag="vb")   # v rows (no bias)
            gqT = bigp.tile([32, Q], BF16, tag="gqT")
            gkT = bigp.tile([32, K], BF16, tag="gkT")
            qT = bigp.tile([128, 4 * NQT, 128], BF16, tag="qT")  # ln(q)^T chunks

            # ---- LayerNorm a group of ntile 128-row tiles in-place ----
            def ln_group(x_g, ntile):
                st6 = stats.tile([128, 8, 6], F32, tag="st6")
                mv = stats.tile([128, 8, 2], F32, tag="mv")
                for t in range(ntile):
                    nc.vector.bn_stats(out=st6[:, t, :], in_=x_g[:, t, :])
                    nc.vector.bn_aggr(out=mv[:, t, :], in_=st6[:, t, :])
                sd = stats.tile([128, 8], F32, tag="sd")
                nc.scalar.activation(out=sd[:, :ntile], in_=mv[:, :ntile, 1],
                                     func=FT.Sqrt, bias=eps, scale=1.0)
                rstd = stats.tile([128, 8], F32, tag="rstd")
                nc.vector.reciprocal(out=rstd[:, :ntile], in_=sd[:, :ntile])
                nmr = stats.tile([128, 8], F32, tag="nmr")
                nc.vector.scalar_tensor_tensor(
                    out=nmr[:, :ntile], in0=mv[:, :ntile, 0], scalar=-1.0,
                    in1=rstd[:, :ntile], op0=ALU.mult, op1=ALU.mult)
                for t in range(ntile):
                    nc.vector.tensor_scalar(
                        out=x_g[:, t, :], in0=x_g[:, t, :],
                        scalar1=rstd[:, t:t + 1], scalar2=nmr[:, t:t + 1],
                        op0=ALU.mult, op1=ALU.add)

            # ---------------- q side: LN + transpose + projections ----------------
            xq = xin.tile([128, NQT, DIM], BF16, tag="xq", bufs=1)
            nc.gpsimd.dma_start(out=xq, in_=q[:, :, :])
            ln_group(xq, NQT)
            for h in range(2):  # two transpose DMAs of 4 tiles each
                nc.sync.dma_start_transpose(
                    out=qT[:, 16 * h:16 * (h + 1), :], in_=xq[:, 4 * h:4 * (h + 1), :])
            qTr = qT.rearrange("p (t c) f -> p c t f", c=4)

            # q_p^T: per m-block, 2 col-groups of 512
            for m in range(4):
                for g in range(2):
                    pp = ps_p.tile([128, 512], F32, tag="pp")
                    for c in range(4):
                        nc.tensor.matmul(pp, wq_sb[:, c, m * 128:(m + 1) * 128],
                                         qTr[:, c, 4 * g:4 * (g + 1), :],
                                         start=(c == 0), stop=(c == 3))
                    nc.scalar.activation(out=qpT[:, m, 512 * g:512 * (g + 1)], in_=pp,
                                         func=FT.Identity, bias=rqt_sb[:, m:m + 1])
            # gate_q^T = WgqS^T @ q_p^T (contraction over projected dim)
            for g in range(2):
                gp = ps_g.tile([32, 512], F32, tag="gq")
                for c in range(4):
                    nc.tensor.matmul(gp, wgq_sb[:, c, :], qpT[:, c, 512 * g:512 * (g + 1)],
                                     start=(c == 0), stop=(c == 3))
                nc.vector.tensor_copy(gqT[:, 512 * g:512 * (g + 1)], gp)

            # ---------------- s side: 4 groups of 8 tiles ----------------
            for g in range(4):
                xs = xin.tile([128, 8, DIM], BF16, tag="xs")
                nc.gpsimd.dma_start(out=xs, in_=s[:, 8 * g:8 * (g + 1), :])
                ln_group(xs, 8)
                sT = xtp.tile([128, 32, 128], BF16, tag="sT")
                nc.sync.dma_start_transpose(out=sT, in_=xs)
                sTr = sT.rearrange("p (t c) f -> p c t f", c=4)
                # k_p^T for these 1024 kv cols
                for m in range(4):
                    for h in range(2):
                        pp = ps_p.tile([128, 512], F32, tag="pp")
                        for c in range(4):
                            nc.tensor.matmul(pp, wk_sb[:, c, m * 128:(m + 1) * 128],
                                             sTr[:, c, 4 * h:4 * (h + 1), :],
                                             start=(c == 0), stop=(c == 3))
                        nc.scalar.activation(
                            out=kTp[:, m, 1024 * g + 512 * h:1024 * g + 512 * (h + 1)],
                            in_=pp, func=FT.Identity, bias=rkt_sb[:, m:m + 1])
                # v rows (no bias; folded into bos)
                for t in range(8):
                    pv = ps_p.tile([128, 512], F32, tag="pp")
                    for c in range(4):
                        nc.tensor.matmul(pv, sTr[:, c, t, :], wv_sb[:, c, :],
                                         start=(c == 0), stop=(c == 3))
                    if t % 2 == 0:
                        nc.vector.tensor_copy(vb[:, 8 * g + t, :], pv)
                    else:
                        nc.scalar.copy(vb[:, 8 * g + t, :], pv)
                # gate_k^T
                for h in range(2):
                    gp = ps_g.tile([32, 512], F32, tag="gq")
                    for c in range(4):
                        nc.tensor.matmul(
                            gp, wgk_sb[:, c, :],
                            kTp[:, c, 1024 * g + 512 * h:1024 * g + 512 * (h + 1)],
                            start=(c == 0), stop=(c == 3))
                    nc.vector.tensor_copy(
                        gkT[:, 1024 * g + 512 * h:1024 * g + 512 * (h + 1)], gp)

            if debug:
                nc.sync.dma_start(out=d_qpT[:, :, :], in_=qpT[:, :, :].bitcast(BF16))
                nc.sync.dma_start(out=d_kT[:, :, :], in_=kTp[:, :, :].bitcast(BF16))
                nc.sync.dma_start(out=d_vb[:, :, :], in_=vb[:, :, :].bitcast(BF16))
                nc.sync.dma_start(out=d_gq[:, :], in_=gqT[:, :].bitcast(BF16))
                nc.sync.dma_start(out=d_gk[:, :], in_=gkT[:, :].bitcast(BF16))

            # ---------------- attention ----------------
            for t in range(NQT):
                w0 = WSTARTS[t]
                qc = bass.ts(t, 128)
                if stage < 2:
                    ob0 = outp.tile([128, DIM], F32, tag="ob")
                    nc.vector.tensor_copy(ob0, vb[:, 4 * t, :])
                    nc.sync.dma_start(out=out[t * 128:(t + 1) * 128, :], in_=ob0)
                    continue
                msk = attn.tile([128, WINW], BF16, tag="msk")
                nc.gpsimd.dma_start(out=msk, in_=m01[t, :, :])
                # gate logits
                gl = ps_b.tile([128, 1024], F32, tag="big")
                for n0 in (0, 512):
                    nn_ = min(512, WINW - n0)
                    nc.tensor.matmul(gl[:, n0:n0 + nn_], gqT[:, qc],
                                     gkT[:, w0 + n0:w0 + n0 + nn_],
                                     start=True, stop=True)
                th = attn.tile([128, WINW], BF16, tag="th")
                nc.scalar.activation(out=th, in_=gl[:, :WINW], func=FT.Tanh, scale=0.5)
                gm = attn.tile([128, WINW], BF16, tag="gm")
                nc.vector.scalar_tensor_tensor(out=gm, in0=th, scalar=1.0, in1=msk,
                                               op0=ALU.add, op1=ALU.mult)
                if stage == 21:
                    ob0 = outp.tile([128, DIM], F32, tag="ob")
                    nc.vector.tensor_copy(ob0, gm[:, :DIM])
                    nc.sync.dma_start(out=out[t * 128:(t + 1) * 128, :], in_=ob0)
                    continue
                # scores
                sc = ps_b.tile([128, 1024], F32, tag="big")
                for n0 in (0, 512):
                    nn_ = min(512, WINW - n0)
                    for c in range(4):
                        nc.tensor.matmul(sc[:, n0:n0 + nn_], qpT[:, c, qc],
                                         kTp[:, c, w0 + n0:w0 + n0 + nn_],
                                         start=(c == 0), stop=(c == 3))
                ex = attn.tile([128, WINW], BF16, tag="ex")
                nc.scalar.activation(out=ex, in_=sc[:, :WINW], func=FT.Exp)
                if stage == 22:
                    ob0 = outp.tile([128, DIM], F32, tag="ob")
                    nc.vector.tensor_copy(ob0, ex[:, :DIM])
                    nc.sync.dma_start(out=out[t * 128:(t + 1) * 128, :], in_=ob0)
                    continue
                P = attn.tile([128, WINW], BF16, tag="P")
                nc.vector.tensor_mul(P, ex, gm)
                rsum = stats.tile([128, 1], F32, tag="rsum")
                nc.vector.tensor_reduce(out=rsum, in_=P, axis=mybir.AxisListType.X,
                                        op=ALU.add)
                rinv = stats.tile([128, 1], F32, tag="rinv")
                nc.vector.reciprocal(out=rinv, in_=rsum)
                if stage < 3:
                    ob0 = outp.tile([128, DIM], F32, tag="ob")
                    nc.vector.tensor_copy(ob0, P[:, :DIM])
                    nc.sync.dma_start(out=out[t * 128:(t + 1) * 128, :], in_=ob0)
                    continue
                PT = attn.tile([128, 6, 128], BF16, tag="PT")
                nc.sync.dma_start_transpose(out=PT, in_=P)
                # attn @ V
                av = ps_p.tile([128, 512], F32, tag="pp")
                for j in range(6):
                    nc.tensor.matmul(av, PT[:, j, :], vb[:, w0 // 128 + j, :],
                                     start=(j == 0), stop=(j == 5))
                oan = outp.tile([128, DIM], BF16, tag="oan")
                nc.scalar.activation(out=oan, in_=av, func=FT.Identity, scale=rinv)
                if debug and t == 0:
                    nc.sync.dma_start(out=d_P[:, :], in_=P.bitcast(BF16))
                    nc.sync.dma_start(out=d_oa[:, :], in_=oan.bitcast(BF16))
                if stage < 4:
                    ob0 = outp.tile([128, DIM], F32, tag="ob")
                    nc.vector.tensor_copy(ob0, av)
                    nc.sync.dma_start(out=out[t * 128:(t + 1) * 128, :], in_=ob0)
                    continue
                oaT = outp.tile([128, 4, 128], BF16, tag="oaT")
                nc.sync.dma_start_transpose(out=oaT, in_=oan)
                # out = oan @ Wo + bos  (bos = rv@Wo + bo)
                fin = ps_p.tile([128, 512], F32, tag="pp")
                for c in range(4):
                    nc.tensor.matmul(fin, oaT[:, c, :], wo_sb[:, c, :],
                                     start=(c == 0), stop=False)
                nc.tensor.matmul(fin, ones1, bos_sb, start=False, stop=True)
                ob = outp.tile([128, DIM], F32, tag="ob")
                nc.vector.tensor_copy(ob, fin)
                nc.sync.dma_start(out=out[t * 128:(t + 1) * 128, :], in_=ob)

    if not nc.is_finalized():
        nc.finalize()
    return nc


_NC_CACHE = {}


def _get_nc(debug=False, stage=4):
    key = (debug, stage)
    if key not in _NC_CACHE:
        _NC_CACHE[key] = build_bass(debug=debug, stage=stage)
    return _NC_CACHE[key]


def _host_fold(inputs):
    f32 = np.float32
    scale = f32(DIM ** -0.5)
    ctx0 = np.asarray(inputs["ctx0"], f32)
    ctx1 = np.asarray(inputs["ctx1"], f32)
    pre = ctx0 @ inputs["Wc0"] + inputs["bc0"] + ctx1 @ inputs["Wc1"] + inputs["bc1"]
    pre = np.asarray(pre, f32)
    h = pre / (1.0 + np.exp(-pre))
    gb = np.asarray(h @ inputs["Wf"] + inputs["bf"], f32)
    gamma, beta = gb[:, :DIM], gb[:, DIM:]

    qn_g = np.asarray(inputs["qn_g"], f32)
    qn_b = np.asarray(inputs["qn_b"], f32)
    kvn_g = np.asarray(inputs["kvn_g"], f32)
    kvn_b = np.asarray(inputs["kvn_b"], f32)
    Wq, bq = np.asarray(inputs["Wq"], f32), np.asarray(inputs["bq"], f32)
    Wk, bk = np.asarray(inputs["Wk"], f32), np.asarray(inputs["bk"], f32)
    Wv, bv = np.asarray(inputs["Wv"], f32), np.asarray(inputs["bv"], f32)
    Wo, bo = np.asarray(inputs["Wo"], f32), np.asarray(inputs["bo"], f32)
    mask = np.asarray(inputs["mask"], f32)

    WkS = np.ascontiguousarray((Wk * kvn_g[:, None]).astype(BFNP))
    r_k = (kvn_b @ Wk + bk).astype(f32)
    WvS = np.ascontiguousarray((Wv * kvn_g[:, None]).astype(BFNP))
    r_v = (kvn_b @ Wv + bv).astype(f32)
    WgqS = np.ascontiguousarray((inputs["Wgq"] / scale / np.sqrt(RANK)).astype(BFNP))
    WgkS = np.ascontiguousarray(np.asarray(inputs["Wgk"], BFNP))
    WoS = np.ascontiguousarray(Wo.astype(BFNP))
    bosv = (r_v @ Wo + bo).astype(BFNP).reshape(1, DIM)

    # multiplicative 0/1 mask (x0.5 folds the tanh->sigmoid affine)
    m01 = np.stack([(mask[t * 128:(t + 1) * 128, w:w + WINW] == 0.0) * 0.5
                    for t, w in enumerate(WSTARTS)]).astype(BFNP)

    query = np.asarray(inputs["query"], f32).reshape(B, Q, DIM)
    source = np.asarray(inputs["source"], f32).reshape(B, K, DIM)
    # device x layout: [128 partitions, tile, 512]
    qdev = np.ascontiguousarray(
        query.reshape(B, NQT, 128, DIM).transpose(0, 2, 1, 3).astype(BFNP))
    sdev = np.ascontiguousarray(
        source.reshape(B, NKT, 128, DIM).transpose(0, 2, 1, 3).astype(BFNP))

    in_maps = []
    for b in range(B):
        sg = (qn_g * (1.0 + gamma[b])).astype(f32)
        WqS = np.ascontiguousarray((Wq * sg[:, None] * scale).astype(BFNP))
        r_q = (((qn_b * (1.0 + gamma[b]) + beta[b]) @ Wq + bq) * scale).astype(f32)
        in_maps.append({
            "q": qdev[b],
            "s": sdev[b],
            "wq": WqS, "wk": WkS, "wv": WvS, "wo": WoS,
            "wgq": WgqS, "wgk": WgkS,
            "rqt": np.ascontiguousarray(r_q.reshape(4, 128).T),
            "rkt": np.ascontiguousarray(r_k.reshape(4, 128).T),
            "bos": bosv,
            "m01": m01,
        })
    return in_maps


def kernel(**inputs):
    nc = _get_nc()
    in_maps = _host_fold(inputs)
    res = run_bass_kernel_spmd(nc, in_maps, core_ids=list(range(B)))
    out = np.stack([res.results[b]["out"] for b in range(B)])
    return out.reshape(B, QS, QT, DIM).astype(np.float32)


if __name__ == "__main__":
    build_bass()
    print("bass build OK")


# revision 15
# speedup vs baseline: 1.5186x; 1.0047x over previous
"""Trainium2 Bass kernel for nn_LocalDenseCrossReadout (v3).

Strategy:
- Data-parallel over batch: 8 batches -> 8 NeuronCores, one batch per core.
- Host-side (numpy, O(D^2)): FiLM conditioning, folding of LayerNorm affine +
  FiLM + score scale into projection weights, v-bias folded into output bias,
  multiplicative 0/1 mask blocks.
- Device, all bf16 matmul inputs (fp32 PSUM accumulate):
  * warmup matmuls on junk data keep the PE HAM clock-gate at K=8/8 while
    LayerNorm runs
  * LayerNorm stats+apply on natural-layout tiles (DVE/ACT split)
  * x^T via hardware DMA-transpose (no PE transposes at all)
  * projections with weights stationary (FWL), 1024-wide PSUM evacuations
    with per-partition bias on the activation engine
  * banded attention, 768-wide window per 128-row q tile:
    gate sigma via tanh (same act table as exp -> no table reloads),
    P = exp(S) * (sigma * mask01), no max-subtraction, P^T and oa^T via
    DMA-transpose, softmax normalization folded into the activation-copy
    scale, v-bias + out-bias as one rank-1 matmul.
- Queues: inputs + outputs on GpSimd (SWDGE), transposes on Sync (HWDGE),
  compute queues never blocked by waiting DMAs.
"""

import sys

sys.path.insert(0, "/opt/trn_rl_repo")

import numpy as np
import ml_dtypes

import concourse.bass as bass
import concourse.tile as tile
from concourse import bacc
from concourse import mybir
from concourse.bass_utils import run_bass_kernel_spmd

DIM, QS, QT, KS, KT, WIN, B, RANK = 512, 64, 16, 256, 16, 4, 8, 32
Q = QS * QT  # 1024
K = KS * KT  # 4096
WINW = 768  # aligned kv window per 128-row q tile
NQT = Q // 128  # 8 q tiles
NKT = K // 128  # 32 kv tiles
F32 = mybir.dt.float32
BF16 = mybir.dt.bfloat16
FT = mybir.ActivationFunctionType
ALU = mybir.AluOpType

BFNP = ml_dtypes.bfloat16

# kv window start (aligned to 128) per q tile
WSTARTS = [0, 384, 896, 1408, 1920, 2432, 2944, 3328]
N_WARM = 48


def build_bass(debug=False, stage=4):
    nc = bacc.Bacc("TRN2", target_bir_lowering=False)
    q = nc.dram_tensor("q", [128, NQT, DIM], BF16, kind="ExternalInput")
    s = nc.dram_tensor("s", [128, NKT, DIM], BF16, kind="ExternalInput")
    wq = nc.dram_tensor("wq", [DIM, DIM], BF16, kind="ExternalInput")
    wk = nc.dram_tensor("wk", [DIM, DIM], BF16, kind="ExternalInput")
    wv = nc.dram_tensor("wv", [DIM, DIM], BF16, kind="ExternalInput")
    wo = nc.dram_tensor("wo", [DIM, DIM], BF16, kind="ExternalInput")
    wgq = nc.dram_tensor("wgq", [DIM, RANK], BF16, kind="ExternalInput")
    wgk = nc.dram_tensor("wgk", [DIM, RANK], BF16, kind="ExternalInput")
    rqt = nc.dram_tensor("rqt", [128, 4], F32, kind="ExternalInput")
    rkt = nc.dram_tensor("rkt", [128, 4], F32, kind="ExternalInput")
    bos = nc.dram_tensor("bos", [1, DIM], BF16, kind="ExternalInput")
    m01 = nc.dram_tensor("m01", [NQT, 128, WINW], BF16, kind="ExternalInput")
    out = nc.dram_tensor("out", [Q, DIM], F32, kind="ExternalOutput")
    wrm = nc.dram_tensor("wrm", [128, 8], F32, kind="ExternalOutput")

    with tile.TileContext(nc) as tc:
        with (
            tc.tile_pool(name="consts", bufs=1) as consts,
            tc.tile_pool(name="wts", bufs=1) as wts,
            tc.tile_pool(name="big", bufs=1) as bigp,
            tc.tile_pool(name="xin", bufs=2) as xin,
            tc.tile_pool(name="xt", bufs=2) as xtp,
            tc.tile_pool(name="stats", bufs=2) as stats,
            tc.tile_pool(name="attn", bufs=2) as attn,
            tc.tile_pool(name="outp", bufs=2) as outp,
            tc.tile_pool(name="ps_big", bufs=2, space="PSUM") as ps_big,
            tc.tile_pool(name="ps_sm", bufs=2, space="PSUM") as ps_sm,
            tc.tile_pool(name="ps_g", bufs=1, space="PSUM") as ps_g,
        ):
            # ---------------- constants + warmup ----------------
            eps = consts.tile([128, 1], F32)
            nc.vector.memset(eps, 1e-5)
            ones1 = consts.tile([1, 128], BF16)
            nc.vector.memset(ones1, 1.0)
            warm_w = consts.tile([128, 128], BF16)
            nc.vector.memset(warm_w, 0.01)
            warm_x = consts.tile([128, 512], BF16)
            nc.vector.memset(warm_x, 0.01)
            for i in range(N_WARM):
                wp = ps_sm.tile([128, 512], F32, tag="sm")
                nc.tensor.matmul(wp, warm_w, warm_x, start=True, stop=True)
                if i == N_WARM - 1:
                    wsb = consts.tile([128, 8], F32)
                    nc.scalar.copy(wsb, wp[:, :8])
                    nc.gpsimd.dma_start(out=wrm[:, :], in_=wsb)

            rqt_sb = consts.tile([128, 4], F32)
            nc.gpsimd.dma_start(out=rqt_sb, in_=rqt[:, :])
            rkt_sb = consts.tile([128, 4], F32)
            nc.gpsimd.dma_start(out=rkt_sb, in_=rkt[:, :])
            bos_sb = consts.tile([1, DIM], BF16)
            nc.gpsimd.dma_start(out=bos_sb, in_=bos[:, :])

            # weights as lhsT chunks: [128 (d_in within chunk c), c, d_out]
            def load_w(name, dram, n_out):
                t = wts.tile([128, 4, n_out], BF16, tag=name)
                for c in range(4):
                    nc.gpsimd.dma_start(out=t[:, c, :], in_=dram[c * 128:(c + 1) * 128, :])
                return t

            wq_sb = load_w("wq", wq, DIM)
            wk_sb = load_w("wk", wk, DIM)
            wv_sb = load_w("wv", wv, DIM)
            wo_sb = load_w("wo", wo, DIM)
            wgq_sb = load_w("wgq", wgq, RANK)
            wgk_sb = load_w("wgk", wgk, RANK)

            # persistent activation tensors
            qpT = bigp.tile([128, 4, Q], BF16, tag="qpT")     # q_p^T d-chunks
            kTp = bigp.tile([128, 4, K], BF16, tag="kTp")     # k_p^T d-chunks
            vb = bigp.tile([128, NKT, DIM], BF16, tag="vb")   # v rows (no bias)
            gqT = bigp.tile([32, Q], BF16, tag="gqT")
            gkT = bigp.tile([32, K], BF16, tag="gkT")
            qT = bigp.tile([128, 4 * NQT, 128], BF16, tag="qT")  # ln(q)^T chunks

            # ---- LayerNorm a group of ntile 128-row tiles in-place.
            # stats/aggr on DVE, sqrt batched on ACT, applies split DVE/ACT.
            def ln_group(x_g, ntile):
                st6 = stats.tile([128, 8, 6], F32, tag="st6")
                mv = stats.tile([128, 8, 2], F32, tag="mv")
                for t in range(ntile):
                    nc.vector.bn_stats(out=st6[:, t, :], in_=x_g[:, t, :])
                    nc.vector.bn_aggr(out=mv[:, t, :], in_=st6[:, t, :])
                sd = stats.tile([128, 8], F32, tag="sd")
                nc.scalar.activation(out=sd[:, :ntile], in_=mv[:, :ntile, 1],
                                     func=FT.Sqrt, bias=eps, scale=1.0)
                rstd = stats.tile([128, 8], F32, tag="rstd")
                nc.vector.reciprocal(out=rstd[:, :ntile], in_=sd[:, :ntile])
                nmr = stats.tile([128, 8], F32, tag="nmr")
                nc.vector.scalar_tensor_tensor(
                    out=nmr[:, :ntile], in0=mv[:, :ntile, 0], scalar=-1.0,
                    in1=rstd[:, :ntile], op0=ALU.mult, op1=ALU.mult)
                for t in range(ntile):
                    if t % 2 == 0:
                        nc.vector.tensor_scalar(
                            out=x_g[:, t, :], in0=x_g[:, t, :],
                            scalar1=rstd[:, t:t + 1], scalar2=nmr[:, t:t + 1],
                            op0=ALU.mult, op1=ALU.add)
                    else:
                        nc.scalar.activation(
                            out=x_g[:, t, :], in_=x_g[:, t, :], func=FT.Identity,
                            scale=rstd[:, t:t + 1], bias=nmr[:, t:t + 1])

            # ---------------- q side: LN + transpose + projections ----------------
            xq = xin.tile([128, NQT, DIM], BF16, tag="xq", bufs=1)
            nc.gpsimd.dma_start(out=xq, in_=q[:, :, :])
            ln_group(xq, NQT)
            for h in range(2):  # two transpose DMAs of 4 tiles each
                nc.sync.dma_start_transpose(
                    out=qT[:, 16 * h:16 * (h + 1), :], in_=xq[:, 4 * h:4 * (h + 1), :])
            qTr = qT.rearrange("p (t c) f -> p c t f", c=4)

            # q_p^T: per m-block, one 1024-wide psum
            for m in range(4):
                pp = ps_big.tile([128, 1024], F32, tag="bigps")
                for g in range(2):
                    for c in range(4):
                        nc.tensor.matmul(pp[:, 512 * g:512 * (g + 1)],
                                         wq_sb[:, c, m * 128:(m + 1) * 128],
                                         qTr[:, c, 4 * g:4 * (g + 1), :],
                                         start=(c == 0), stop=(c == 3))
                nc.scalar.activation(out=qpT[:, m, :], in_=pp,
                                     func=FT.Identity, bias=rqt_sb[:, m:m + 1])
            # gate_q^T = WgqS^T @ q_p^T (contraction over projected dim)
            for g in range(2):
                gp = ps_g.tile([32, 512], F32, tag="gq")
                for c in range(4):
                    nc.tensor.matmul(gp, wgq_sb[:, c, :], qpT[:, c, 512 * g:512 * (g + 1)],
                                     start=(c == 0), stop=(c == 3))
                nc.vector.tensor_copy(gqT[:, 512 * g:512 * (g + 1)], gp)

            # ---------------- s side: 4 groups of 8 tiles ----------------
            for g in range(4):
                xs = xin.tile([128, 8, DIM], BF16, tag="xs")
                nc.gpsimd.dma_start(out=xs, in_=s[:, 8 * g:8 * (g + 1), :])
                ln_group(xs, 8)
                sT = xtp.tile([128, 32, 128], BF16, tag="sT")
                nc.sync.dma_start_transpose(out=sT, in_=xs)
                sTr = sT.rearrange("p (t c) f -> p c t f", c=4)
                # k_p^T for these 1024 kv cols
                for m in range(4):
                    pp = ps_big.tile([128, 1024], F32, tag="bigps")
                    for h in range(2):
                        for c in range(4):
                            nc.tensor.matmul(pp[:, 512 * h:512 * (h + 1)],
                                             wk_sb[:, c, m * 128:(m + 1) * 128],
                                             sTr[:, c, 4 * h:4 * (h + 1), :],
                                             start=(c == 0), stop=(c == 3))
                    nc.scalar.activation(out=kTp[:, m, 1024 * g:1024 * (g + 1)],
                                         in_=pp, func=FT.Identity,
                                         bias=rkt_sb[:, m:m + 1])
                # v rows (no bias; folded into bos), two tiles per psum
                for u in range(4):
                    pv = ps_big.tile([128, 1024], F32, tag="bigps")
                    for h in range(2):
                        for c in range(4):
                            nc.tensor.matmul(pv[:, 512 * h:512 * (h + 1)],
                                             sTr[:, c, 2 * u + h, :], wv_sb[:, c, :],
                                             start=(c == 0), stop=(c == 3))
                    nc.scalar.copy(vb[:, 8 * g + 2 * u:8 * g + 2 * u + 2, :],
                                   pv.rearrange("p (a b) -> p a b", a=2))
                # gate_k^T
                for h in range(2):
                    gp = ps_g.tile([32, 512], F32, tag="gq")
                    for c in range(4):
                        nc.tensor.matmul(
                            gp, wgk_sb[:, c, :],
                            kTp[:, c, 1024 * g + 512 * h:1024 * g + 512 * (h + 1)],
                            start=(c == 0), stop=(c == 3))
                    nc.vector.tensor_copy(
                        gkT[:, 1024 * g + 512 * h:1024 * g + 512 * (h + 1)], gp)

            # ---------------- attention ----------------
            msks = []
            for t in range(NQT):
                msk = attn.tile([128, WINW], BF16, tag="msk", bufs=4)
                nc.gpsimd.dma_start(out=msk, in_=m01[t, :, :])
                msks.append(msk)
            for t in range(NQT):
                w0 = WSTARTS[t]
                qc = bass.ts(t, 128)
                if stage < 2:
                    ob0 = outp.tile([128, DIM], F32, tag="ob")
                    nc.vector.tensor_copy(ob0, vb[:, 4 * t, :])
                    nc.sync.dma_start(out=out[t * 128:(t + 1) * 128, :], in_=ob0)
                    continue
                msk = msks[t]
                # gate logits
                gl = ps_big.tile([128, 1024], F32, tag="bigps")
                for n0 in (0, 512):
                    nn_ = min(512, WINW - n0)
                    nc.tensor.matmul(gl[:, n0:n0 + nn_], gqT[:, qc],
                                     gkT[:, w0 + n0:w0 + n0 + nn_],
                                     start=True, stop=True)
                th = attn.tile([128, WINW], BF16, tag="th")
                nc.scalar.activation(out=th, in_=gl[:, :WINW], func=FT.Tanh, scale=0.5)
                gm = attn.tile([128, WINW], BF16, tag="gm")
                nc.vector.scalar_tensor_tensor(out=gm, in0=th, scalar=1.0, in1=msk,
                                               op0=ALU.add, op1=ALU.mult)
                # scores
                sc = ps_big.tile([128, 1024], F32, tag="bigps")
                for n0 in (0, 512):
                    nn_ = min(512, WINW - n0)
                    for c in range(4):
                        nc.tensor.matmul(sc[:, n0:n0 + nn_], qpT[:, c, qc],
                                         kTp[:, c, w0 + n0:w0 + n0 + nn_],
                                         start=(c == 0), stop=(c == 3))
                ex = attn.tile([128, WINW], BF16, tag="ex")
                nc.scalar.activation(out=ex, in_=sc[:, :WINW], func=FT.Exp)
                P = attn.tile([128, WINW], BF16, tag="P")
                nc.vector.tensor_mul(P, ex, gm)
                rsum = stats.tile([128, 1], F32, tag="rsum")
                nc.vector.tensor_reduce(out=rsum, in_=P, axis=mybir.AxisListType.X,
                                        op=ALU.add)
                rinv = stats.tile([128, 1], F32, tag="rinv")
                nc.vector.reciprocal(out=rinv, in_=rsum)
                PT = attn.tile([128, 6, 128], BF16, tag="PT")
                nc.sync.dma_start_transpose(out=PT, in_=P)
                # attn @ V
                av = ps_sm.tile([128, 512], F32, tag="sm")
                for j in range(6):
                    nc.tensor.matmul(av, PT[:, j, :], vb[:, w0 // 128 + j, :],
                                     start=(j == 0), stop=(j == 5))
                oan = outp.tile([128, DIM], BF16, tag="oan")
                nc.scalar.activation(out=oan, in_=av, func=FT.Identity, scale=rinv)
                oaT = outp.tile([128, 4, 128], BF16, tag="oaT")
                nc.sync.dma_start_transpose(out=oaT, in_=oan)
                # out = oan @ Wo + bos  (bos = rv@Wo + bo)
                fin = ps_sm.tile([128, 512], F32, tag="sm")
                for c in range(4):
                    nc.tensor.matmul(fin, oaT[:, c, :], wo_sb[:, c, :],
                                     start=(c == 0), stop=False)
                nc.tensor.matmul(fin, ones1, bos_sb, start=False, stop=True)
                ob = outp.tile([128, DIM], F32, tag="ob")
                if t % 2 == 0:
                    nc.vector.tensor_copy(ob, fin)
                else:
                    nc.scalar.copy(ob, fin)
                nc.gpsimd.dma_start(out=out[t * 128:(t + 1) * 128, :], in_=ob)

    if not nc.is_finalized():
        nc.finalize()
    return nc


_NC_CACHE = {}


def _get_nc(debug=False, stage=4):
    key = (debug, stage)
    if key not in _NC_CACHE:
        _NC_CACHE[key] = build_bass(debug=debug, stage=stage)
    return _NC_CACHE[key]


def _host_fold(inputs):
    f32 = np.float32
    scale = f32(DIM ** -0.5)
    ctx0 = np.asarray(inputs["ctx0"], f32)
    ctx1 = np.asarray(inputs["ctx1"], f32)
    pre = ctx0 @ inputs["Wc0"] + inputs["bc0"] + ctx1 @ inputs["Wc1"] + inputs["bc1"]
    pre = np.asarray(pre, f32)
    h = pre / (1.0 + np.exp(-pre))
    gb = np.asarray(h @ inputs["Wf"] + inputs["bf"], f32)
    gamma, beta = gb[:, :DIM], gb[:, DIM:]

    qn_g = np.asarray(inputs["qn_g"], f32)
    qn_b = np.asarray(inputs["qn_b"], f32)
    kvn_g = np.asarray(inputs["kvn_g"], f32)
    kvn_b = np.asarray(inputs["kvn_b"], f32)
    Wq, bq = np.asarray(inputs["Wq"], f32), np.asarray(inputs["bq"], f32)
    Wk, bk = np.asarray(inputs["Wk"], f32), np.asarray(inputs["bk"], f32)
    Wv, bv = np.asarray(inputs["Wv"], f32), np.asarray(inputs["bv"], f32)
    Wo, bo = np.asarray(inputs["Wo"], f32), np.asarray(inputs["bo"], f32)
    mask = np.asarray(inputs["mask"], f32)

    WkS = np.ascontiguousarray((Wk * kvn_g[:, None]).astype(BFNP))
    r_k = (kvn_b @ Wk + bk).astype(f32)
    WvS = np.ascontiguousarray((Wv * kvn_g[:, None]).astype(BFNP))
    r_v = (kvn_b @ Wv + bv).astype(f32)
    WgqS = np.ascontiguousarray((inputs["Wgq"] / scale / np.sqrt(RANK)).astype(BFNP))
    WgkS = np.ascontiguousarray(np.asarray(inputs["Wgk"], BFNP))
    WoS = np.ascontiguousarray(Wo.astype(BFNP))
    bosv = (r_v @ Wo + bo).astype(BFNP).reshape(1, DIM)

    # multiplicative 0/1 mask (x0.5 folds the tanh->sigmoid affine)
    m01 = np.stack([(mask[t * 128:(t + 1) * 128, w:w + WINW] == 0.0) * 0.5
                    for t, w in enumerate(WSTARTS)]).astype(BFNP)

    query = np.asarray(inputs["query"], f32).reshape(B, Q, DIM)
    source = np.asarray(inputs["source"], f32).reshape(B, K, DIM)
    # device x layout: [128 partitions, tile, 512]
    qdev = np.ascontiguousarray(
        query.reshape(B, NQT, 128, DIM).transpose(0, 2, 1, 3).astype(BFNP))
    sdev = np.ascontiguousarray(
        source.reshape(B, NKT, 128, DIM).transpose(0, 2, 1, 3).astype(BFNP))

    in_maps = []
    for b in range(B):
        sg = (qn_g * (1.0 + gamma[b])).astype(f32)
        WqS = np.ascontiguousarray((Wq * sg[:, None] * scale).astype(BFNP))
        r_q = (((qn_b * (1.0 + gamma[b]) + beta[b]) @ Wq + bq) * scale).astype(f32)
        in_maps.append({
            "q": qdev[b],
            "s": sdev[b],
            "wq": WqS, "wk": WkS, "wv": WvS, "wo": WoS,
            "wgq": WgqS, "wgk": WgkS,
            "rqt": np.ascontiguousarray(r_q.reshape(4, 128).T),
            "rkt": np.ascontiguousarray(r_k.reshape(4, 128).T),
            "bos": bosv,
            "m01": m01,
        })
    return in_maps


def kernel(**inputs):
    nc = _get_nc()
    in_maps = _host_fold(inputs)
    res = run_bass_kernel_spmd(nc, in_maps, core_ids=list(range(B)))
    out = np.stack([res.results[b]["out"] for b in range(B)])
    return out.reshape(B, QS, QT, DIM).astype(np.float32)


if __name__ == "__main__":
    build_bass()
    print("bass build OK")


# revision 17
# speedup vs baseline: 1.8023x; 1.1868x over previous
"""Trainium2 Bass kernel for nn_LocalDenseCrossReadout (v3).

Strategy:
- Data-parallel over batch: 8 batches -> 8 NeuronCores, one batch per core.
- Host-side (numpy, O(D^2)): FiLM conditioning, folding of LayerNorm affine +
  FiLM + score scale into projection weights, v-bias folded into output bias,
  multiplicative 0/1 mask blocks.
- Device, all bf16 matmul inputs (fp32 PSUM accumulate):
  * warmup matmuls on junk data keep the PE HAM clock-gate at K=8/8 while
    LayerNorm runs
  * LayerNorm stats+apply on natural-layout tiles (DVE/ACT split)
  * x^T via hardware DMA-transpose (no PE transposes at all)
  * projections with weights stationary (FWL), 1024-wide PSUM evacuations
    with per-partition bias on the activation engine
  * banded attention, 768-wide window per 128-row q tile:
    gate sigma via tanh (same act table as exp -> no table reloads),
    P = exp(S) * (sigma * mask01), no max-subtraction, P^T and oa^T via
    DMA-transpose, softmax normalization folded into the activation-copy
    scale, v-bias + out-bias as one rank-1 matmul.
- Queues: inputs + outputs on GpSimd (SWDGE), transposes on Sync (HWDGE),
  compute queues never blocked by waiting DMAs.
"""

import sys

sys.path.insert(0, "/opt/trn_rl_repo")

import numpy as np
import ml_dtypes

import concourse.bass as bass
import concourse.tile as tile
from concourse import bacc
from concourse import mybir
from concourse.bass_utils import run_bass_kernel_spmd

DIM, QS, QT, KS, KT, WIN, B, RANK = 512, 64, 16, 256, 16, 4, 8, 32
Q = QS * QT  # 1024
K = KS * KT  # 4096
WINW = 768  # aligned kv window per 128-row q tile
NQT = Q // 128  # 8 q tiles
NKT = K // 128  # 32 kv tiles
F32 = mybir.dt.float32
BF16 = mybir.dt.bfloat16
FT = mybir.ActivationFunctionType
ALU = mybir.AluOpType

BFNP = ml_dtypes.bfloat16

# kv window start (aligned to 128) per q tile
WSTARTS = [0, 384, 896, 1408, 1920, 2432, 2944, 3328]
N_WARM = 48


def build_bass(debug=False, stage=4):
    nc = bacc.Bacc("TRN2", target_bir_lowering=False)
    q = nc.dram_tensor("q", [128, NQT, DIM], BF16, kind="ExternalInput")
    s = nc.dram_tensor("s", [128, NKT, DIM], BF16, kind="ExternalInput")
    wq = nc.dram_tensor("wq", [DIM, DIM], BF16, kind="ExternalInput")
    wk = nc.dram_tensor("wk", [DIM, DIM], BF16, kind="ExternalInput")
    wv = nc.dram_tensor("wv", [DIM, DIM], BF16, kind="ExternalInput")
    wo = nc.dram_tensor("wo", [DIM, DIM], BF16, kind="ExternalInput")
    wgq = nc.dram_tensor("wgq", [DIM, RANK], BF16, kind="ExternalInput")
    wgk = nc.dram_tensor("wgk", [DIM, RANK], BF16, kind="ExternalInput")
    rqt = nc.dram_tensor("rqt", [128, 4], F32, kind="ExternalInput")
    rkt = nc.dram_tensor("rkt", [128, 4], F32, kind="ExternalInput")
    bos = nc.dram_tensor("bos", [1, DIM], BF16, kind="ExternalInput")
    m01 = nc.dram_tensor("m01", [NQT, 128, WINW], BF16, kind="ExternalInput")
    out = nc.dram_tensor("out", [Q, DIM], F32, kind="ExternalOutput")
    wrm = nc.dram_tensor("wrm", [128, 8], F32, kind="ExternalOutput")

    with tile.TileContext(nc) as tc:
        with (
            tc.tile_pool(name="consts", bufs=1) as consts,
            tc.tile_pool(name="wts", bufs=1) as wts,
            tc.tile_pool(name="big", bufs=1) as bigp,
            tc.tile_pool(name="xin", bufs=2) as xin,
            tc.tile_pool(name="xt", bufs=2) as xtp,
            tc.tile_pool(name="stats", bufs=2) as stats,
            tc.tile_pool(name="attn", bufs=2) as attn,
            tc.tile_pool(name="outp", bufs=2) as outp,
            tc.tile_pool(name="ps_big", bufs=2, space="PSUM") as ps_big,
            tc.tile_pool(name="ps_sm", bufs=2, space="PSUM") as ps_sm,
            tc.tile_pool(name="ps_g", bufs=1, space="PSUM") as ps_g,
        ):
            # ---------------- constants + warmup ----------------
            eps = consts.tile([128, 1], F32)
            nc.vector.memset(eps, 1e-5)
            ones1 = consts.tile([1, 128], BF16)
            nc.vector.memset(ones1, 1.0)
            warm_w = consts.tile([128, 128], BF16)
            nc.vector.memset(warm_w, 0.01)
            warm_x = consts.tile([128, 512], BF16)
            nc.vector.memset(warm_x, 0.01)
            for i in range(N_WARM):
                wp = ps_sm.tile([128, 512], F32, tag="sm")
                nc.tensor.matmul(wp, warm_w, warm_x, start=True, stop=True)
                if i == N_WARM - 1:
                    wsb = consts.tile([128, 8], F32)
                    nc.scalar.copy(wsb, wp[:, :8])
                    nc.gpsimd.dma_start(out=wrm[:, :], in_=wsb)

            rqt_sb = consts.tile([128, 4], F32)
            nc.gpsimd.dma_start(out=rqt_sb, in_=rqt[:, :])
            rkt_sb = consts.tile([128, 4], F32)
            nc.gpsimd.dma_start(out=rkt_sb, in_=rkt[:, :])
            bos_sb = consts.tile([1, DIM], BF16)
            nc.gpsimd.dma_start(out=bos_sb, in_=bos[:, :])

            # weights as lhsT chunks: [128 (d_in within chunk c), c, d_out]
            def load_w(name, dram, n_out):
                t = wts.tile([128, 4, n_out], BF16, tag=name)
                nc.gpsimd.dma_start(
                    out=t, in_=dram.rearrange("(c p) d -> p c d", p=128))
                return t

            wq_sb = load_w("wq", wq, DIM)
            wk_sb = load_w("wk", wk, DIM)
            wv_sb = load_w("wv", wv, DIM)
            wo_sb = load_w("wo", wo, DIM)
            wgq_sb = load_w("wgq", wgq, RANK)
            wgk_sb = load_w("wgk", wgk, RANK)

            # persistent activation tensors
            qpT = bigp.tile([128, 4, Q], BF16, tag="qpT")     # q_p^T d-chunks
            kTp = bigp.tile([128, 4, K], BF16, tag="kTp")     # k_p^T d-chunks
            vb = bigp.tile([128, NKT, DIM], BF16, tag="vb")   # v rows (no bias)
            gqT = bigp.tile([32, Q], BF16, tag="gqT")
            gkT = bigp.tile([32, K], BF16, tag="gkT")
            qT = bigp.tile([128, 4 * NQT, 128], BF16, tag="qT")  # ln(q)^T chunks

            # ---- LayerNorm a group of ntile 128-row tiles in-place.
            # stats/aggr on DVE, sqrt batched on ACT, applies split DVE/ACT.
            def ln_group(x_g, ntile):
                st6 = stats.tile([128, 8, 6], F32, tag="st6")
                mv = stats.tile([128, 8, 2], F32, tag="mv")
                for t in range(ntile):
                    nc.vector.bn_stats(out=st6[:, t, :], in_=x_g[:, t, :])
                    nc.vector.bn_aggr(out=mv[:, t, :], in_=st6[:, t, :])
                sd = stats.tile([128, 8], F32, tag="sd")
                nc.scalar.activation(out=sd[:, :ntile], in_=mv[:, :ntile, 1],
                                     func=FT.Sqrt, bias=eps, scale=1.0)
                rstd = stats.tile([128, 8], F32, tag="rstd")
                nc.vector.reciprocal(out=rstd[:, :ntile], in_=sd[:, :ntile])
                nmr = stats.tile([128, 8], F32, tag="nmr")
                nc.vector.scalar_tensor_tensor(
                    out=nmr[:, :ntile], in0=mv[:, :ntile, 0], scalar=-1.0,
                    in1=rstd[:, :ntile], op0=ALU.mult, op1=ALU.mult)
                for t in range(ntile):
                    if t % 2 == 0:
                        nc.vector.tensor_scalar(
                            out=x_g[:, t, :], in0=x_g[:, t, :],
                            scalar1=rstd[:, t:t + 1], scalar2=nmr[:, t:t + 1],
                            op0=ALU.mult, op1=ALU.add)
                    else:
                        nc.scalar.activation(
                            out=x_g[:, t, :], in_=x_g[:, t, :], func=FT.Identity,
                            scale=rstd[:, t:t + 1], bias=nmr[:, t:t + 1])

            # ---------------- q side: LN + transpose + projections ----------------
            xq = xin.tile([128, NQT, DIM], BF16, tag="xq", bufs=1)
            nc.sync.dma_start(out=xq, in_=q[:, :, :])
            ln_group(xq, NQT)
            for h in range(2):  # two transpose DMAs of 4 tiles each
                nc.sync.dma_start_transpose(
                    out=qT[:, 16 * h:16 * (h + 1), :], in_=xq[:, 4 * h:4 * (h + 1), :])
            qTr = qT.rearrange("p (t c) f -> p c t f", c=4)

            # q_p^T: per m-block, one 1024-wide psum
            for m in range(4):
                pp = ps_big.tile([128, 1024], F32, tag="bigps")
                for g in range(2):
                    for c in range(4):
                        nc.tensor.matmul(pp[:, 512 * g:512 * (g + 1)],
                                         wq_sb[:, c, m * 128:(m + 1) * 128],
                                         qTr[:, c, 4 * g:4 * (g + 1), :],
                                         start=(c == 0), stop=(c == 3))
                nc.scalar.activation(out=qpT[:, m, :], in_=pp,
                                     func=FT.Identity, bias=rqt_sb[:, m:m + 1])
            # gate_q^T = WgqS^T @ q_p^T (contraction over projected dim)
            for g in range(2):
                gbig = ps_g.tile([128, 1024], F32, tag="g")
                gp = gbig[0:32, 0:512]
                for c in range(4):
                    nc.tensor.matmul(gp, wgq_sb[:, c, :], qpT[:, c, 512 * g:512 * (g + 1)],
                                     start=(c == 0), stop=(c == 3))
                nc.vector.tensor_copy(gqT[:, 512 * g:512 * (g + 1)], gp)

            # ---------------- s side: 4 groups of 8 tiles ----------------
            for g in range(4):
                xs = xin.tile([128, 8, DIM], BF16, tag="xs")
                nc.sync.dma_start(out=xs, in_=s[:, 8 * g:8 * (g + 1), :])
                ln_group(xs, 8)
                sT = xtp.tile([128, 32, 128], BF16, tag="sT")
                nc.sync.dma_start_transpose(out=sT, in_=xs)
                sTr = sT.rearrange("p (t c) f -> p c t f", c=4)
                # k_p^T for these 1024 kv cols
                for m in range(4):
                    pp = ps_big.tile([128, 1024], F32, tag="bigps")
                    for h in range(2):
                        for c in range(4):
                            nc.tensor.matmul(pp[:, 512 * h:512 * (h + 1)],
                                             wk_sb[:, c, m * 128:(m + 1) * 128],
                                             sTr[:, c, 4 * h:4 * (h + 1), :],
                                             start=(c == 0), stop=(c == 3))
                    nc.scalar.activation(out=kTp[:, m, 1024 * g:1024 * (g + 1)],
                                         in_=pp, func=FT.Identity,
                                         bias=rkt_sb[:, m:m + 1])
                # v rows (no bias; folded into bos), two tiles per psum
                for u in range(4):
                    pv = ps_big.tile([128, 1024], F32, tag="bigps")
                    for h in range(2):
                        for c in range(4):
                            nc.tensor.matmul(pv[:, 512 * h:512 * (h + 1)],
                                             sTr[:, c, 2 * u + h, :], wv_sb[:, c, :],
                                             start=(c == 0), stop=(c == 3))
                    nc.scalar.copy(vb[:, 8 * g + 2 * u:8 * g + 2 * u + 2, :],
                                   pv.rearrange("p (a b) -> p a b", a=2))
                # gate_k^T
                for h in range(2):
                    gbig = ps_g.tile([128, 1024], F32, tag="g")
                    gp = gbig[0:32, 0:512]
                    for c in range(4):
                        nc.tensor.matmul(
                            gp, wgk_sb[:, c, :],
                            kTp[:, c, 1024 * g + 512 * h:1024 * g + 512 * (h + 1)],
                            start=(c == 0), stop=(c == 3))
                    nc.vector.tensor_copy(
                        gkT[:, 1024 * g + 512 * h:1024 * g + 512 * (h + 1)], gp)

            # ---------------- attention (software-pipelined, lookahead 2) ----
            msks = []
            for t in range(NQT):
                msk = attn.tile([128, WINW], BF16, tag="msk", bufs=4)
                nc.gpsimd.dma_start(out=msk, in_=m01[t, :, :])
                msks.append(msk)

            LOOK = 2
            state = {}

            def attn_front(t):
                w0 = WSTARTS[t]
                qc = bass.ts(t, 128)
                # gate logits in ps_g (shared with projection-phase gq/gk)
                gl = ps_g.tile([128, 1024], F32, tag="g")
                for n0 in (0, 512):
                    nn_ = min(512, WINW - n0)
                    nc.tensor.matmul(gl[:, n0:n0 + nn_], gqT[:, qc],
                                     gkT[:, w0 + n0:w0 + n0 + nn_],
                                     start=True, stop=True)
                th = attn.tile([128, WINW], BF16, tag="th", bufs=3)
                nc.scalar.activation(out=th, in_=gl[:, :WINW], func=FT.Tanh, scale=0.5)
                gm = attn.tile([128, WINW], BF16, tag="gm", bufs=3)
                nc.vector.scalar_tensor_tensor(out=gm, in0=th, scalar=1.0,
                                               in1=msks[t], op0=ALU.add, op1=ALU.mult)
                # scores
                sc = ps_big.tile([128, 1024], F32, tag="bigps")
                for n0 in (0, 512):
                    nn_ = min(512, WINW - n0)
                    for c in range(4):
                        nc.tensor.matmul(sc[:, n0:n0 + nn_], qpT[:, c, qc],
                                         kTp[:, c, w0 + n0:w0 + n0 + nn_],
                                         start=(c == 0), stop=(c == 3))
                ex = attn.tile([128, WINW], BF16, tag="ex", bufs=3)
                nc.scalar.activation(out=ex, in_=sc[:, :WINW], func=FT.Exp)
                P = attn.tile([128, WINW], BF16, tag="P", bufs=3)
                nc.vector.tensor_mul(P, ex, gm)
                rsum = stats.tile([128, 1], F32, tag="rsum", bufs=4)
                nc.vector.tensor_reduce(out=rsum, in_=P, axis=mybir.AxisListType.X,
                                        op=ALU.add)
                rinv = stats.tile([128, 1], F32, tag="rinv", bufs=4)
                nc.vector.reciprocal(out=rinv, in_=rsum)
                PT = attn.tile([128, 6, 128], BF16, tag="PT", bufs=3)
                nc.sync.dma_start_transpose(out=PT, in_=P)
                state[t] = (PT, rinv)

            def attn_back(t):
                w0 = WSTARTS[t]
                PT, rinv = state.pop(t)
                av = ps_sm.tile([128, 512], F32, tag="sm")
                for j in range(6):
                    nc.tensor.matmul(av, PT[:, j, :], vb[:, w0 // 128 + j, :],
                                     start=(j == 0), stop=(j == 5))
                oan = outp.tile([128, DIM], BF16, tag="oan")
                nc.scalar.activation(out=oan, in_=av, func=FT.Identity, scale=rinv)
                oaT = outp.tile([128, 4, 128], BF16, tag="oaT")
                nc.scalar.dma_start_transpose(out=oaT, in_=oan)
                fin = ps_sm.tile([128, 512], F32, tag="sm")
                for c in range(4):
                    nc.tensor.matmul(fin, oaT[:, c, :], wo_sb[:, c, :],
                                     start=(c == 0), stop=False)
                nc.tensor.matmul(fin, ones1, bos_sb, start=False, stop=True)
                ob = outp.tile([128, DIM], F32, tag="ob")
                if t % 2 == 0:
                    nc.vector.tensor_copy(ob, fin)
                else:
                    nc.scalar.copy(ob, fin)
                nc.gpsimd.dma_start(out=out[t * 128:(t + 1) * 128, :], in_=ob)

            for t in range(NQT + LOOK):
                if t < NQT:
                    attn_front(t)
                if t >= LOOK:
                    attn_back(t - LOOK)

    if not nc.is_finalized():
        nc.finalize()
    return nc


_NC_CACHE = {}


def _get_nc(debug=False, stage=4):
    key = (debug, stage)
    if key not in _NC_CACHE:
        _NC_CACHE[key] = build_bass(debug=debug, stage=stage)
    return _NC_CACHE[key]


def _host_fold(inputs):
    f32 = np.float32
    scale = f32(DIM ** -0.5)
    ctx0 = np.asarray(inputs["ctx0"], f32)
    ctx1 = np.asarray(inputs["ctx1"], f32)
    pre = ctx0 @ inputs["Wc0"] + inputs["bc0"] + ctx1 @ inputs["Wc1"] + inputs["bc1"]
    pre = np.asarray(pre, f32)
    h = pre / (1.0 + np.exp(-pre))
    gb = np.asarray(h @ inputs["Wf"] + inputs["bf"], f32)
    gamma, beta = gb[:, :DIM], gb[:, DIM:]

    qn_g = np.asarray(inputs["qn_g"], f32)
    qn_b = np.asarray(inputs["qn_b"], f32)
    kvn_g = np.asarray(inputs["kvn_g"], f32)
    kvn_b = np.asarray(inputs["kvn_b"], f32)
    Wq, bq = np.asarray(inputs["Wq"], f32), np.asarray(inputs["bq"], f32)
    Wk, bk = np.asarray(inputs["Wk"], f32), np.asarray(inputs["bk"], f32)
    Wv, bv = np.asarray(inputs["Wv"], f32), np.asarray(inputs["bv"], f32)
    Wo, bo = np.asarray(inputs["Wo"], f32), np.asarray(inputs["bo"], f32)
    mask = np.asarray(inputs["mask"], f32)

    WkS = np.ascontiguousarray((Wk * kvn_g[:, None]).astype(BFNP))
    r_k = (kvn_b @ Wk + bk).astype(f32)
    WvS = np.ascontiguousarray((Wv * kvn_g[:, None]).astype(BFNP))
    r_v = (kvn_b @ Wv + bv).astype(f32)
    WgqS = np.ascontiguousarray((inputs["Wgq"] / scale / np.sqrt(RANK)).astype(BFNP))
    WgkS = np.ascontiguousarray(np.asarray(inputs["Wgk"], BFNP))
    WoS = np.ascontiguousarray(Wo.astype(BFNP))
    bosv = (r_v @ Wo + bo).astype(BFNP).reshape(1, DIM)

    # multiplicative 0/1 mask (x0.5 folds the tanh->sigmoid affine)
    m01 = np.stack([(mask[t * 128:(t + 1) * 128, w:w + WINW] == 0.0) * 0.5
                    for t, w in enumerate(WSTARTS)]).astype(BFNP)

    query = np.asarray(inputs["query"], f32).reshape(B, Q, DIM)
    source = np.asarray(inputs["source"], f32).reshape(B, K, DIM)
    # device x layout: [128 partitions, tile, 512]
    qdev = np.ascontiguousarray(
        query.reshape(B, NQT, 128, DIM).transpose(0, 2, 1, 3).astype(BFNP))
    sdev = np.ascontiguousarray(
        source.reshape(B, NKT, 128, DIM).transpose(0, 2, 1, 3).astype(BFNP))

    in_maps = []
    for b in range(B):
        sg = (qn_g * (1.0 + gamma[b])).astype(f32)
        WqS = np.ascontiguousarray((Wq * sg[:, None] * scale).astype(BFNP))
        r_q = (((qn_b * (1.0 + gamma[b]) + beta[b]) @ Wq + bq) * scale).astype(f32)
        in_maps.append({
            "q": qdev[b],
            "s": sdev[b],
            "wq": WqS, "wk": WkS, "wv": WvS, "wo": WoS,
            "wgq": WgqS, "wgk": WgkS,
            "rqt": np.ascontiguousarray(r_q.reshape(4, 128).T),
            "rkt": np.ascontiguousarray(r_k.reshape(4, 128).T),
            "bos": bosv,
            "m01": m01,
        })
    return in_maps


def kernel(**inputs):
    nc = _get_nc()
    in_maps = _host_fold(inputs)
    res = run_bass_kernel_spmd(nc, in_maps, core_ids=list(range(B)))
    out = np.stack([res.results[b]["out"] for b in range(B)])
    return out.reshape(B, QS, QT, DIM).astype(np.float32)


if __name__ == "__main__":
    build_bass()
    print("bass build OK")
